# revision 1
# baseline (speedup 1.0000x reference)
"""LocallyConnectedXYZLayer Trainium2 kernel.

out[n,c,h,w] = sum_{dy,dx in 5x5} exp(-|xyz(n,:,h+dy-2,w+dx-2)-xyz(n,:,h,w)|^2/2)
               * (softmax*mask)(n,c,h+dy-2,w+dx-2)        (zero-padded)

Sharding: 8 cores = (batch n = core//2) x (W half = core%2).
Per-core layout: partitions = 2 w-chunks x 64 h rows; free dims = (dy, c, w).
The 5 dy window shifts are baked into host-prepared per-partition rows (one
DMA per tensor per step); dx shifts are free-dim slices.  Per 5x5 offset:
one bf16 tensor_tensor mul (gaussian broadcast over channels via a step-0 AP
dim) + one accumulate add on the vector engine; squared-distance chain runs
in fp32 on gpsimd, exp on the scalar engine; per-dx group sums in bf16 with
an fp32 master accumulator (hierarchical accumulation for precision).
"""

import sys
from contextlib import ExitStack

import numpy as np

sys.path.insert(0, "/opt/trn_rl_repo")

import ml_dtypes  # noqa: E402

import concourse.bass as bass  # noqa: E402
from concourse import mybir  # noqa: E402
from concourse.bass_utils import run_bass_kernel_spmd  # noqa: E402

BF16 = ml_dtypes.bfloat16

N, C, H, W = 4, 20, 64, 2048
KH = KW = 5
PAD = 2
HH = H + 2 * PAD  # 68 padded rows
WCORE = W // 2  # 1024 interior w per core
NSTEP = 4  # device steps
WS = WCORE // (2 * NSTEP)  # 128 interior w per (step, chunk)
WX = WS + 2 * PAD  # 132 w extent (halo 2 each side)

_CACHE = {}


def _build_nc():
    """Raw-Bass program (no Tile): this toolchain's walrus codegen allows at
    most one sync-wait command per instruction, so all cross-engine sync is
    standalone wait_ge instructions plus one then_inc on producer ops."""
    nc = bass.Bass("TRN2", target_bir_lowering=False, debug=False)
    bf = mybir.dt.bfloat16
    f32 = mybir.dt.float32
    sm_d = nc.dram_tensor("sm_in", [NSTEP, 128, 2, KH, C, WX], bf,
                          kind="ExternalInput")
    xyz_d = nc.dram_tensor("xyz_in", [NSTEP, 128, KH, 3, WX], bf,
                           kind="ExternalInput")
    m_d = nc.dram_tensor("m_in", [NSTEP, 128, KH, WX], bf,
                         kind="ExternalInput")
    out_d = nc.dram_tensor("out_d", [NSTEP, 128, C, WS], f32,
                           kind="ExternalOutput")

    def sb(name, shape, dt):
        return nc.alloc_sbuf_tensor(name, list(shape), dt).ap()

    # double-buffered input tiles (per step parity)
    sm_t = [sb(f"sm{i}", [128, 2, KH, C, WX], bf) for i in range(2)]
    xyz_t = [sb(f"xyz{i}", [128, KH, 3, WX], bf) for i in range(2)]
    m_t = [sb(f"m{i}", [128, KH, WX], bf) for i in range(2)]
    # d2 chain (gpsimd-internal reuse is in-order; d2 double-buffered for ACT)
    diff_t = sb("diff", [128, KH, 3, WS], bf)
    d2_t = [sb(f"d2_{i}", [128, KH, WS], bf) for i in range(2)]
    g5_t = [sb(f"g5_{i}", [128, KH, WS], bf) for i in range(2)]
    gm5_t = sb("gm5", [128, KH, WS], bf)
    t_t = sb("t", [128, C, WS], bf)
    group_t = sb("group", [128, C, WS], bf)
    master_t = [sb(f"master{i}", [128, C, WS], f32) for i in range(2)]

    ADD, MULT, SUB = (mybir.AluOpType.add, mybir.AluOpType.mult,
                      mybir.AluOpType.subtract)

    with ExitStack() as ctx:
        load_sem = ctx.enter_context(nc.semaphore("load_sem"))
        sm_sem = ctx.enter_context(nc.semaphore("sm_sem"))
        sme_sem = ctx.enter_context(nc.semaphore("sme_sem"))
        store_sem = ctx.enter_context(nc.semaphore("store_sem"))
        pool_sem = ctx.enter_context(nc.semaphore("pool_sem"))
        act_sem = ctx.enter_context(nc.semaphore("act_sem"))
        dve_sem = ctx.enter_context(nc.semaphore("dve_sem"))
        block = ctx.enter_context(nc.Block())

        @block.sync
        def _(sync):
            for s in range(NSTEP):
                b = s % 2
                if s >= 1:
                    # DMA completions across steps are unordered; gate this
                    # step's loads on the previous step's completions so the
                    # cumulative load_sem threshold implies the right data.
                    sync.wait_ge(load_sem, 32 * s)
                    sync.wait_ge(sm_sem, 16 * s)
                    sync.wait_ge(sme_sem, 16 * s)
                if s >= 2:
                    # input buffer reuse: step s-2 consumers must be done
                    # (master of tau=5(s-2)+4; all reads precede it)
                    sync.wait_ge(dve_sem, 15 * (5 * s - 6) + 19)
                sync.dma_start(xyz_t[b][:], xyz_d[s]).then_inc(load_sem, 16)
                sync.dma_start(m_t[b][:], m_d[s]).then_inc(load_sem, 16)
                sync.dma_start(sm_t[b][:, 0], sm_d[s, :, 0]).then_inc(sme_sem, 16)
                sync.dma_start(sm_t[b][:, 1], sm_d[s, :, 1]).then_inc(sm_sem, 16)
                if s >= 1:
                    sync.wait_ge(dve_sem, 15 * (5 * s - 1) + 19)
                    if s >= 2:
                        sync.wait_ge(store_sem, 16 * (s - 1))
                    sync.dma_start(out_d[s - 1],
                                   master_t[(s - 1) % 2][:]).then_inc(
                                       store_sem, 16)
            sync.wait_ge(dve_sem, 300)
            sync.wait_ge(store_sem, 16 * (NSTEP - 1))
            sync.dma_start(out_d[NSTEP - 1],
                           master_t[(NSTEP - 1) % 2][:]).then_inc(
                               store_sem, 16)

        @block.scalar
        def _(scalar):
            for s in range(NSTEP):
                for dx in range(KW):
                    tau = 5 * s + dx
                    scalar.wait_ge(dve_sem, 4 if tau == 0 else 15 * tau - 7)
                    if tau >= 2:
                        # g5 buffer reuse: gm5 of group tau-2 must be done
                        scalar.wait_ge(dve_sem, 15 * (tau - 2) + 9)
                    scalar.activation(
                        out=g5_t[tau % 2][:], in_=d2_t[tau % 2][:],
                        func=mybir.ActivationFunctionType.Exp,
                        scale=-0.5).then_inc(act_sem)

        @block.vector
        def _(vector):
            # Everything data-parallel lives on DVE (gpsimd sharing the SBUF
            # ports slowed concurrent DVE ops ~3x, a worse trade than doing
            # the d2 chain here at 2x bf16).  The d2 chain for tau+1 is
            # emitted before the MAC of tau so ACT's exp overlaps the MAC.
            nv = [0]

            def vop(bi):
                nv[0] += 1
                return bi

            def vwait():
                if nv[0] > 0:
                    vector.wait_ge(dve_sem, nv[0])

            def d2chain(tau):
                sp, dxp = divmod(tau, KW)
                bp = sp % 2
                if dxp == 0:
                    # xyz + m suffice for the d2 chain and gm5
                    vector.wait_ge(load_sem, 32 * (sp + 1))
                if tau >= 2:
                    # d2 buffer reuse: exp of group tau-2 must be done
                    vector.wait_ge(act_sem, tau - 1)
                xyz_c = xyz_t[bp][:, 2, :, PAD:PAD + WS].unsqueeze(
                    1).broadcast_to([128, KH, 3, WS])
                vwait()
                vop(vector.tensor_tensor(
                    out=diff_t[:], in0=xyz_t[bp][:, :, :, dxp:dxp + WS],
                    in1=xyz_c, op=SUB).then_inc(dve_sem))
                vwait()
                vop(vector.tensor_tensor(
                    out=diff_t[:], in0=diff_t[:], in1=diff_t[:],
                    op=MULT).then_inc(dve_sem))
                d2 = d2_t[tau % 2]
                vwait()
                vop(vector.tensor_tensor(
                    out=d2[:], in0=diff_t[:, :, 0, :], in1=diff_t[:, :, 1, :],
                    op=ADD).then_inc(dve_sem))
                vwait()
                vop(vector.tensor_tensor(
                    out=d2[:], in0=d2[:], in1=diff_t[:, :, 2, :],
                    op=ADD).then_inc(dve_sem))

            d2chain(0)
            for tau in range(NSTEP * KW):
                s, dx = divmod(tau, KW)
                b = s % 2
                if dx == 0 and s >= 2:
                    # master buffer reuse: store of step s-2 must be done
                    vector.wait_ge(store_sem, 16 * (s - 1))
                master = master_t[b]
                if tau + 1 < NSTEP * KW:
                    d2chain(tau + 1)
                vector.wait_ge(act_sem, tau + 1)
                if dx == 0:
                    vector.wait_ge(sme_sem, 16 * (s + 1))
                elif dx == 1:
                    vector.wait_ge(sm_sem, 16 * (s + 1))
                vwait()
                vop(vector.tensor_tensor(
                    out=gm5_t[:], in0=g5_t[tau % 2][:],
                    in1=m_t[b][:, :, dx:dx + WS],
                    op=MULT).then_inc(dve_sem))
                e, off = dx % 2, dx - dx % 2
                for dy in range(KH):
                    sm_s = sm_t[b][:, e, dy, :, off:off + WS]
                    g_b = gm5_t[:, dy, :].unsqueeze(1).broadcast_to(
                        [128, C, WS])
                    if dy == 0:
                        vwait()
                        vop(vector.tensor_tensor(
                            out=group_t[:], in0=sm_s, in1=g_b,
                            op=MULT).then_inc(dve_sem))
                    else:
                        vwait()
                        vop(vector.tensor_tensor(
                            out=t_t[:], in0=sm_s, in1=g_b,
                            op=MULT).then_inc(dve_sem))
                        vwait()
                        vop(vector.tensor_tensor(
                            out=group_t[:], in0=group_t[:], in1=t_t[:],
                            op=ADD).then_inc(dve_sem))
                vwait()
                if dx == 0:
                    vop(vector.tensor_copy(
                        master[:], group_t[:]).then_inc(dve_sem))
                else:
                    vop(vector.tensor_tensor(
                        out=master[:], in0=master[:], in1=group_t[:],
                        op=ADD).then_inc(dve_sem))
            assert nv[0] == 300, nv[0]

    return nc


def _prep_core(xyz, softmax, mask, core):
    """Build the per-core dy-baked slab arrays (host side, bf16).

    Row layout: partition p (0..127) = chunk (p//64) x h row (p%64); the
    dy dim holds the 5 shifted window rows h+dy (in padded coords)."""
    n, half = core // 2, core % 2
    w0 = WCORE * half
    wp_sz = WCORE + 2 * PAD + 1
    lo, hi = w0 - PAD, w0 + WCORE + PAD + 1
    glo, ghi = max(lo, 0), min(hi, W)

    smp = np.zeros((HH, C, wp_sz), BF16)
    smp[PAD:PAD + H, :, glo - lo:ghi - lo] = (
        softmax[n][:, :, glo:ghi].transpose(1, 0, 2).astype(BF16))
    xyzp = np.zeros((HH, 3, wp_sz), BF16)
    xyzp[PAD:PAD + H, :, glo - lo:ghi - lo] = (
        xyz[n][:, :, glo:ghi].transpose(1, 0, 2).astype(BF16))
    mp = np.zeros((HH, wp_sz), BF16)
    mp[PAD:PAD + H, glo - lo:ghi - lo] = mask[n][:, glo:ghi].astype(BF16)

    sm5 = np.empty((NSTEP, 128, 2, KH, C, WX), BF16)
    xyz5 = np.empty((NSTEP, 128, KH, 3, WX), BF16)
    m5 = np.empty((NSTEP, 128, KH, WX), BF16)
    for s in range(NSTEP):
        for chunk in range(2):
            wb = WS * (s + NSTEP * chunk)
            pr = slice(64 * chunk, 64 * chunk + 64)
            for dy in range(KH):
                for e in range(2):
                    sm5[s, pr, e, dy] = smp[dy:dy + 64, :, wb + e:wb + e + WX]
                xyz5[s, pr, dy] = xyzp[dy:dy + 64, :, wb:wb + WX]
                m5[s, pr, dy] = mp[dy:dy + 64, wb:wb + WX]
    return {"sm_in": sm5, "xyz_in": xyz5, "m_in": m5}


def make_in_maps(xyz, softmax, mask):
    return [_prep_core(xyz, softmax, mask, k) for k in range(8)]


def assemble_out(results):
    out = np.empty((N, C, H, W), np.float32)
    for core in range(8):
        n, half = core // 2, core % 2
        w0 = WCORE * half
        o = np.asarray(results[core]["out_d"], dtype=np.float32)
        # [s, chunk*64+h, c, j] -> [c, h, (s + NSTEP*chunk)*WS + j]
        o = o.reshape(NSTEP, 2, H, C, WS)
        # -> [c, h, chunk, s, j]
        out[n, :, :, w0:w0 + WCORE] = o.transpose(3, 2, 1, 0, 4).reshape(
            C, H, WCORE)
    return out


def get_nc():
    if "nc" not in _CACHE:
        _CACHE["nc"] = _build_nc()
    return _CACHE["nc"]


def kernel(xyz, softmax, mask, trace=False, trace_kwargs=None):
    nc = get_nc()
    in_maps = make_in_maps(np.asarray(xyz), np.asarray(softmax),
                           np.asarray(mask))
    res = run_bass_kernel_spmd(nc, in_maps, list(range(8)), trace=trace,
                               **(trace_kwargs or {}))
    out = assemble_out(res.results)
    if trace:
        return out, res
    return out



# revision 5
# speedup vs baseline: 1.5449x; 1.5449x over previous
"""LocallyConnectedXYZLayer Trainium2 kernel.

out[n,c,h,w] = sum_{dy,dx in 5x5} exp(-|xyz(n,:,h+dy-2,w+dx-2)-xyz(n,:,h,w)|^2/2)
               * (softmax*mask)(n,c,h+dy-2,w+dx-2)        (zero-padded)

Sharding: 8 cores = (batch n = core//2) x (W half = core%2).
Per-core layout: partitions = 2 w-chunks x 64 h rows; free dims = (dy, c, w).
The 5 dy window shifts are baked into host-prepared per-partition rows; dx
shifts are free-dim slices (an e-parity duplicate of the softmax slab keeps
every slice 4B-aligned so DVE ops run in the 2x dual-pump mode).

Engine split per (step s, dx) tau:
  Pool : diff = xyz_slab[dx] - xyz_center            (bf16 tensor_tensor sub)
         d2   = sq0 + sq1; d2 += sq2                 (adds)
  ACT  : sq   = Square(diff);  g5 = Exp(-d2/2)
  DVE  : P[dy,c,w] = g5[dy,w] (bcast over c) * sm_slab[e,dy,c,w+off]
         (the ONLY DVE op: one bf16 2x tensor_tensor per tau)
  PE   : PSUM[c,w] += I128 @ P[dy]  for the 25 (dy,dx) planes of a step
         (identity matmuls accumulate in fp32 PSUM, 3 banks per parity)
  ACT  : evac PSUM -> SBUF bf16; DMA out (host casts to f32).
"""

import sys
from contextlib import ExitStack

import numpy as np

sys.path.insert(0, "/opt/trn_rl_repo")

import ml_dtypes  # noqa: E402

import concourse.bass as bass  # noqa: E402
from concourse import mybir  # noqa: E402
from concourse.bass_utils import run_bass_kernel_spmd  # noqa: E402

BF16 = ml_dtypes.bfloat16

N, C, H, W = 4, 20, 64, 2048
KH = KW = 5
PAD = 2
HH = H + 2 * PAD  # 68 padded rows
WCORE = W // 2  # 1024 interior w per core
NSTEP = 8  # device steps
WS = WCORE // (2 * NSTEP)  # 64 interior w per (step, chunk)
WX = WS + 2 * PAD  # 68 w extent (halo 2 each side)
NTAU = NSTEP * KW  # 40
CW = C * WS  # 1280 psum accumulator columns
# matmul column chunks (each within one 512-f32 psum bank): c-row ranges
MM_CHUNKS = [(0, 8), (8, 8), (16, 4)]  # (c0, cn): cols = cn*WS = 512,512,256
MM_PER_TAU = KH * len(MM_CHUNKS)  # 15
MM_PER_STEP = KW * MM_PER_TAU  # 75

_CACHE = {}


def _build_nc():
    """Raw-Bass program (no Tile): this toolchain's walrus codegen allows at
    most one sync-wait command per instruction, so all cross-engine sync is
    standalone wait_ge instructions plus one then_inc on producer ops."""
    nc = bass.Bass("TRN2", target_bir_lowering=False, debug=False)
    bf = mybir.dt.bfloat16
    f32 = mybir.dt.float32
    sm_d = nc.dram_tensor("sm_in", [NSTEP, 128, 2, KH, C, WX], bf,
                          kind="ExternalInput")
    xyz_d = nc.dram_tensor("xyz_in", [NSTEP, 128, KH, 3, WX], bf,
                           kind="ExternalInput")
    id_d = nc.dram_tensor("id_in", [128, 128], bf, kind="ExternalInput")
    out_d = nc.dram_tensor("out_d", [NSTEP, 128, C, WS], bf,
                           kind="ExternalOutput")

    def sb(name, shape, dt):
        return nc.alloc_sbuf_tensor(name, list(shape), dt).ap()

    sm_t = [sb(f"sm{i}", [128, 2, KH, C, WX], bf) for i in range(2)]
    xyz_t = [sb(f"xyz{i}", [128, KH, 3, WX], bf) for i in range(2)]
    id_t = sb("ident", [128, 128], bf)
    diff_t = [sb(f"diff{i}", [128, KH, 3, WS], bf) for i in range(2)]
    sq_t = [sb(f"sq{i}", [128, KH, 3, WS], bf) for i in range(2)]
    d2_t = [sb(f"d2_{i}", [128, KH, WS], bf) for i in range(2)]
    g5_t = [sb(f"g5_{i}", [128, KH, WS], bf) for i in range(2)]
    p_t = [sb(f"p{i}", [128, KH, C, WS], bf) for i in range(2)]
    ob_t = [sb(f"ob{i}", [128, C, WS], bf) for i in range(2)]
    ps_t = [nc.alloc_psum_tensor(f"ps{i}", [128, 3 * 512], f32).ap()
            for i in range(2)]

    ADD, MULT, SUB = (mybir.AluOpType.add, mybir.AluOpType.mult,
                      mybir.AluOpType.subtract)
    AF = mybir.ActivationFunctionType

    with ExitStack() as ctx:
        load_sem = ctx.enter_context(nc.semaphore("load_sem"))  # xyz loads
        sme_sem = ctx.enter_context(nc.semaphore("sme_sem"))  # sm e=0 loads
        smo_sem = ctx.enter_context(nc.semaphore("smo_sem"))  # sm e=1 loads
        id_sem = ctx.enter_context(nc.semaphore("id_sem"))  # identity load
        store_sem = ctx.enter_context(nc.semaphore("store_sem"))
        sub_sem = ctx.enter_context(nc.semaphore("sub_sem"))  # pool sub done
        pool_sem = ctx.enter_context(nc.semaphore("pool_sem"))  # pool op count
        d2_sem = ctx.enter_context(nc.semaphore("d2_sem"))  # pool d2 done
        sq_sem = ctx.enter_context(nc.semaphore("sq_sem"))  # act square done
        exp_sem = ctx.enter_context(nc.semaphore("exp_sem"))  # act exp done
        mul_sem = ctx.enter_context(nc.semaphore("mul_sem"))  # dve mul done
        pe_sem = ctx.enter_context(nc.semaphore("pe_sem"))  # pe matmuls done
        evac_sem = ctx.enter_context(nc.semaphore("evac_sem"))  # act evac done
        block = ctx.enter_context(nc.Block())

        @block.sync
        def _(sync):
            sync.dma_start(id_t[:], id_d[:]).then_inc(id_sem, 16)
            for s in range(NSTEP):
                b = s % 2
                if s >= 2:
                    # input buffer reuse: step s-2 consumers must be done
                    sync.wait_ge(sub_sem, KW * (s - 1))
                    sync.wait_ge(mul_sem, KW * (s - 1))
                if s >= 1:
                    # DMA completions across steps are unordered; gate this
                    # step's loads on the previous step's completions so the
                    # cumulative thresholds imply the right data landed.
                    sync.wait_ge(load_sem, 16 * s)
                    sync.wait_ge(sme_sem, 16 * s)
                    sync.wait_ge(smo_sem, 16 * s)
                sync.dma_start(xyz_t[b][:], xyz_d[s]).then_inc(load_sem, 16)
                sync.dma_start(sm_t[b][:, 0], sm_d[s, :, 0]).then_inc(
                    sme_sem, 16)
                sync.dma_start(sm_t[b][:, 1], sm_d[s, :, 1]).then_inc(
                    smo_sem, 16)
                if s >= 1:
                    # store step s-1 once its evacuation is done
                    sync.wait_ge(evac_sem, s)
                    if s >= 2:
                        sync.wait_ge(store_sem, 16 * (s - 1))
                    sync.dma_start(out_d[s - 1],
                                   ob_t[(s - 1) % 2][:]).then_inc(
                                       store_sem, 16)
            sync.wait_ge(evac_sem, NSTEP)
            sync.wait_ge(store_sem, 16 * (NSTEP - 1))
            sync.dma_start(out_d[NSTEP - 1],
                           ob_t[(NSTEP - 1) % 2][:]).then_inc(store_sem, 16)

        @block.gpsimd
        def _(gpsimd):
            for tau in range(NTAU):
                s, dxp = divmod(tau, KW)
                b, t2 = s % 2, tau % 2
                # ---- diff = xyz_slab[:, :, :, dxp:dxp+WS] - center ----
                if dxp == 0:
                    gpsimd.wait_ge(load_sem, 16 * (s + 1))
                if tau >= 2:
                    # diff buffer reuse: ACT square of tau-2 must be done
                    gpsimd.wait_ge(sq_sem, tau - 1)
                xyz_c = xyz_t[b][:, 2, :, PAD:PAD + WS].unsqueeze(
                    1).broadcast_to([128, KH, 3, WS])
                gpsimd.tensor_tensor(
                    out=diff_t[t2][:], in0=xyz_t[b][:, :, :, dxp:dxp + WS],
                    in1=xyz_c, op=SUB).then_inc(sub_sem)
                # ---- d2 = sq0 + sq1 + sq2 ----
                gpsimd.wait_ge(sq_sem, tau + 1)
                if tau >= 2:
                    # d2 buffer reuse: ACT exp of tau-2 must be done
                    gpsimd.wait_ge(exp_sem, tau - 1)
                gpsimd.tensor_tensor(
                    out=d2_t[t2][:], in0=sq_t[t2][:, :, 0, :],
                    in1=sq_t[t2][:, :, 1, :], op=ADD).then_inc(pool_sem)
                # same-engine RAW on d2: engine queue is FIFO but SBUF writes
                # are pipelined; wait for the previous op's commit.
                gpsimd.wait_ge(pool_sem, tau + 1)
                gpsimd.tensor_tensor(
                    out=d2_t[t2][:], in0=d2_t[t2][:],
                    in1=sq_t[t2][:, :, 2, :], op=ADD).then_inc(d2_sem)

        @block.scalar
        def _(scalar):
            for tau in range(NTAU):
                s, dxp = divmod(tau, KW)
                t2 = tau % 2
                # ---- sq = Square(diff) ----
                scalar.wait_ge(sub_sem, tau + 1)
                if tau >= 2:
                    # sq buffer reuse: pool d2 of tau-2 must be done
                    scalar.wait_ge(d2_sem, tau - 1)
                scalar.activation(out=sq_t[t2][:], in_=diff_t[t2][:],
                                  func=AF.Square).then_inc(sq_sem)
                # ---- evacuate psum of step s-2 (placed after sq of the
                # second tau of each step so PE has had a step of slack) ----
                if dxp == 1 and tau > KW:
                    sev = (tau - KW - 1) // KW  # step being evacuated
                    scalar.wait_ge(pe_sem, MM_PER_STEP * (sev + 1))
                    if sev >= 2:
                        # ob buffer reuse: store of step sev-2 must be done
                        scalar.wait_ge(store_sem, 16 * (sev - 1))
                    scalar.activation(out=ob_t[sev % 2][:],
                                      in_=ps_t[sev % 2][:, 0:CW],
                                      func=AF.Copy).then_inc(evac_sem)
                # ---- g5 = exp(-d2/2) ----
                scalar.wait_ge(d2_sem, tau + 1)
                if tau >= 2:
                    # g5 buffer reuse: DVE mul of tau-2 must be done
                    scalar.wait_ge(mul_sem, tau - 1)
                scalar.activation(out=g5_t[t2][:], in_=d2_t[t2][:],
                                  func=AF.Exp, scale=-0.5).then_inc(exp_sem)
            # final evacuation (the in-loop evacs cover steps 0..NSTEP-2)
            for sev in (NSTEP - 1,):
                scalar.wait_ge(pe_sem, MM_PER_STEP * (sev + 1))
                scalar.wait_ge(store_sem, 16 * (sev - 1))
                scalar.activation(out=ob_t[sev % 2][:],
                                  in_=ps_t[sev % 2][:, 0:CW],
                                  func=AF.Copy).then_inc(evac_sem)

        @block.vector
        def _(vector):
            for tau in range(NTAU):
                s, dxp = divmod(tau, KW)
                b, t2 = s % 2, tau % 2
                e, off = dxp % 2, dxp - dxp % 2
                vector.wait_ge(exp_sem, tau + 1)
                if dxp == 0:
                    vector.wait_ge(sme_sem, 16 * (s + 1))
                elif dxp == 1:
                    vector.wait_ge(smo_sem, 16 * (s + 1))
                if tau >= 2:
                    # p buffer reuse: PE matmuls of tau-2 must be done
                    vector.wait_ge(pe_sem, MM_PER_TAU * (tau - 1))
                g_b = g5_t[t2][:].unsqueeze(2).broadcast_to([128, KH, C, WS])
                vector.tensor_tensor(
                    out=p_t[t2][:], in0=g_b,
                    in1=sm_t[b][:, e, :, :, off:off + WS],
                    op=MULT).then_inc(mul_sem)

        @block.tensor
        def _(tensor):
            tensor.wait_ge(id_sem, 16)
            for tau in range(NTAU):
                s, dxp = divmod(tau, KW)
                t2 = tau % 2
                tensor.wait_ge(mul_sem, tau + 1)
                if dxp == 0 and s >= 2:
                    # psum parity reuse: evacuation of step s-2 must be done
                    tensor.wait_ge(evac_sem, s - 1)
                ps = ps_t[s % 2]
                nmm = 0
                for dy in range(KH):
                    for c0, cn in MM_CHUNKS:
                        nmm += 1
                        mm = tensor.matmul(
                            ps[:, c0 * WS:(c0 + cn) * WS],
                            id_t[:],
                            p_t[t2][:, dy, c0:c0 + cn, :],
                            start=(dxp == 0 and dy == 0),
                            stop=(dxp == KW - 1 and dy == KH - 1),
                            skip_group_check=True,
                        )
                        if nmm == MM_PER_TAU:
                            mm.then_inc(pe_sem, MM_PER_TAU)

    return nc


def _prep_core(xyz, softmax, mask, core):
    """Build the per-core dy-baked slab arrays (host side, bf16).

    Row layout: partition p (0..127) = chunk (p//64) x h row (p%64); the
    dy dim holds the 5 shifted window rows h+dy (in padded coords)."""
    n, half = core // 2, core % 2
    w0 = WCORE * half
    wp_sz = WCORE + 2 * PAD + 1
    lo, hi = w0 - PAD, w0 + WCORE + PAD + 1
    glo, ghi = max(lo, 0), min(hi, W)

    smm = (softmax[n][:, :, glo:ghi]
           * mask[n][None, :, glo:ghi].astype(np.float32))
    smp = np.zeros((HH, C, wp_sz), BF16)
    smp[PAD:PAD + H, :, glo - lo:ghi - lo] = smm.transpose(1, 0, 2).astype(
        BF16)
    xyzp = np.zeros((HH, 3, wp_sz), BF16)
    xyzp[PAD:PAD + H, :, glo - lo:ghi - lo] = (
        xyz[n][:, :, glo:ghi].transpose(1, 0, 2).astype(BF16))

    sm5 = np.empty((NSTEP, 128, 2, KH, C, WX), BF16)
    xyz5 = np.empty((NSTEP, 128, KH, 3, WX), BF16)
    for s in range(NSTEP):
        for chunk in range(2):
            wb = WS * s + (WCORE // 2) * chunk
            pr = slice(64 * chunk, 64 * chunk + 64)
            for dy in range(KH):
                for e in range(2):
                    sm5[s, pr, e, dy] = smp[dy:dy + 64, :, wb + e:wb + e + WX]
                xyz5[s, pr, dy] = xyzp[dy:dy + 64, :, wb:wb + WX]
    ident = np.eye(128, dtype=BF16)
    return {"sm_in": sm5, "xyz_in": xyz5, "id_in": ident}


def make_in_maps(xyz, softmax, mask):
    return [_prep_core(xyz, softmax, mask, k) for k in range(8)]


def assemble_out(results):
    out = np.empty((N, C, H, W), np.float32)
    for core in range(8):
        n, half = core // 2, core % 2
        w0 = WCORE * half
        o = np.asarray(results[core]["out_d"]).astype(np.float32)
        # [s, chunk*64+h, c, j] -> [c, h, WS*s + 512*chunk + j]
        o = o.reshape(NSTEP, 2, H, C, WS)
        # -> [c, h, chunk, s, j]
        out[n, :, :, w0:w0 + WCORE] = o.transpose(3, 2, 1, 0, 4).reshape(
            C, H, WCORE)
    return out


def get_nc():
    if "nc" not in _CACHE:
        _CACHE["nc"] = _build_nc()
    return _CACHE["nc"]


def kernel(xyz, softmax, mask, trace=False, trace_kwargs=None):
    nc = get_nc()
    in_maps = make_in_maps(np.asarray(xyz), np.asarray(softmax),
                           np.asarray(mask))
    res = run_bass_kernel_spmd(nc, in_maps, list(range(8)), trace=trace,
                               **(trace_kwargs or {}))
    out = assemble_out(res.results)
    if trace:
        return out, res
    return out
